# revision 1
# baseline (speedup 1.0000x reference)
"""Block-sparse attention (block-diagonal mask, full-row softmax) on 8 trn2 cores.

Reference semantics (B=1, H=16, S=4096, D=64, BLOCK=64):
    scores  = (Q @ K^T) / 8                     [S, S] per head
    scores *= blockdiag_mask                    (off-block -> 0, NOT -inf)
    weights = softmax(scores, axis=-1)          (over the FULL row)
    out     = weights @ V

Off-block entries contribute exp(0)=1 to the softmax, so for row q in
block b:
    num_q   = sum_{k in b} e_qk v_k - V_bsum(b) + V_total
    denom_q = sum_{k in b} e_qk - 64 + S
    out_q   = num_q / denom_q
Only the diagonal 64x64 blocks are ever materialized.

Sharding: 16 heads over 8 cores -> 2 heads/core, no cross-core comms.

Per-core pipeline (iteration = pair of 128-row chunks; all-bf16 matmuls):
  - fp32 loads are streamed in 8-chunk slabs (HWDGE); GpSimd casts Q/K
    slabs to bf16, DVE casts V.
  - 4 per-chunk PE transposes -> [64(d), 128(s)] into one PSUM bank; one
    DVE copy drops them into a fixed [66, 512] staging tile whose rows
    64:66 hold constant "mask rows" that add -M^2 to every cross-block
    score, so exp underflows to exact 0 off-block.
  - Per chunk: one 66-partition scores matmul -> [128, 128] PSUM (own
    bank - matmul outputs must sit at bank offset 0 on HW); one exp;
    then E^T @ [V|1], a block-diagonal -1 matmul (-bsum), and a rank-1
    [Vtot|S] accumulate num|denom; reciprocal + per-partition scale.
"""

import numpy as np

H, S, D = 16, 4096, 64
HPC = 2  # heads per core
NCORES = 8
CHUNK = 128
NCHUNK = S // CHUNK  # 32
NPAIR = NCHUNK // 2  # 16
NSLAB = 4
SLABC = NCHUNK // NSLAB  # 8 chunks per slab
SCALE = 0.125  # 1/sqrt(D)
MASK_M = 64.0  # M^2*SCALE = 512: exp underflows to exact 0

_CACHE = {}


def _build_bass():
    import concourse.bass as bass
    import concourse.bacc as bacc
    import concourse.tile as tile
    from concourse import mybir
    from concourse.masks import make_identity

    f32 = mybir.dt.float32
    bf16 = mybir.dt.bfloat16
    EXP = mybir.ActivationFunctionType.Exp
    COPY = mybir.ActivationFunctionType.Copy

    nc = bacc.Bacc(
        "TRN2", target_bir_lowering=False, debug=False, num_devices=NCORES
    )
    q_d = nc.dram_tensor("query", [HPC, S, D], f32, kind="ExternalInput")
    k_d = nc.dram_tensor("key", [HPC, S, D], f32, kind="ExternalInput")
    v_d = nc.dram_tensor("value", [HPC, S, D], f32, kind="ExternalInput")
    o_d = nc.dram_tensor("out", [HPC, S, D], f32, kind="ExternalOutput")

    NT = 4  # fixed staging tiles (mask rows written once)

    with tile.TileContext(nc) as tc:
        with (
            tc.tile_pool(name="consts", bufs=1) as consts,
            tc.tile_pool(name="heads", bufs=2) as heads,
            tc.tile_pool(name="work", bufs=6) as work,
            tc.tile_pool(name="vt", bufs=2) as vtp,
            tc.tile_pool(name="ps_t", bufs=2, space="PSUM") as ps_t,
            tc.tile_pool(name="ps_s", bufs=3, space="PSUM") as ps_s,
            tc.tile_pool(name="ps_o", bufs=3, space="PSUM") as ps_o,
        ):
            identb = consts.tile([128, 128], bf16, tag="identb")
            make_identity(nc, identb)
            ones_row = consts.tile([1, 128], bf16, tag="ones_row")
            nc.gpsimd.memset(ones_row, 1.0)
            ones_col = consts.tile([128, 1], f32, tag="ones_col")
            nc.gpsimd.memset(ones_col, 1.0)

            # Block-diagonal -1 (the "-bsum" correction as a matmul weight)
            negblk = consts.tile([128, 128], bf16, tag="negblk")
            nc.gpsimd.memset(negblk, 0.0)
            nc.gpsimd.memset(negblk[0:64, 0:64], -1.0)
            nc.gpsimd.memset(negblk[64:128, 64:128], -1.0)

            # Fixed transpose-staging tiles [66, 512] bf16:
            # cols [Qc 0:128 | Qc1 128:256 | Kc 256:384 | Kc1 384:512],
            # rows 64:66 = mask rows (written once):
            #   Q side: -M where (r + jb) == 1   (jb = 64-col parity)
            #   K side: +M where  r == jb
            tsbs = []
            for i in range(NT):
                t = consts.tile([66, 512], bf16, tag=f"tsb{i}")
                nc.gpsimd.memset(t[64:66, :], 0.0)
                nc.gpsimd.affine_select(
                    out=t[64:66, 0:256].rearrange("p (w b j) -> p w b j", w=2, b=2),
                    in_=t[64:66, 0:256].rearrange("p (w b j) -> p w b j", w=2, b=2),
                    compare_op=mybir.AluOpType.not_equal,
                    fill=-MASK_M,
                    base=-1,
                    pattern=[[0, 2], [1, 2], [0, 64]],
                    channel_multiplier=1,
                )
                nc.gpsimd.affine_select(
                    out=t[64:66, 256:512].rearrange("p (w b j) -> p w b j", w=2, b=2),
                    in_=t[64:66, 256:512].rearrange("p (w b j) -> p w b j", w=2, b=2),
                    compare_op=mybir.AluOpType.not_equal,
                    fill=MASK_M,
                    base=0,
                    pattern=[[0, 2], [-1, 2], [0, 64]],
                    channel_multiplier=1,
                )
                tsbs.append(t)

            for h in range(HPC):
                qh = heads.tile([128, NCHUNK, D], f32, tag="qh")
                kh = heads.tile([128, NCHUNK, D], f32, tag="kh")
                vh = heads.tile([128, NCHUNK, D + 1], f32, tag="vh")
                oh = heads.tile([128, NCHUNK, D], f32, tag="oh")
                qhb = heads.tile([128, NCHUNK, D], bf16, tag="qhb")
                khb = heads.tile([128, NCHUNK, D], bf16, tag="khb")
                vhb = heads.tile([128, NCHUNK, D + 1], bf16, tag="vhb")

                def slab_dma(dst, src, s):
                    nc.sync.dma_start(
                        out=dst[:, s * SLABC : (s + 1) * SLABC, :],
                        in_=src.rearrange("(c p) d -> p c d", p=128)[
                            :, s * SLABC : (s + 1) * SLABC, :
                        ],
                    )

                # V slabs first (Vtot gates the first EV), then Q/K slabs.
                vdst = vh[:, :, 0:D]
                for s in range(NSLAB):
                    slab_dma(vdst, v_d[h], s)
                for s in range(NSLAB):
                    slab_dma(qh, q_d[h], s)
                    slab_dma(kh, kh_d[h] if False else k_d[h], s)

                nc.vector.memset(vh[:, :, D : D + 1], 1.0)

                # slab casts: V on DVE (fast 2x mode), Q/K on GpSimd (idle)
                for s in range(NSLAB):
                    sl = slice(s * SLABC, (s + 1) * SLABC)
                    nc.vector.tensor_copy(out=vhb[:, sl, :], in_=vh[:, sl, :])
                for s in range(NSLAB):
                    sl = slice(s * SLABC, (s + 1) * SLABC)
                    nc.gpsimd.tensor_copy(out=qhb[:, sl, :], in_=qh[:, sl, :])
                    nc.gpsimd.tensor_copy(out=khb[:, sl, :], in_=kh[:, sl, :])

                # V_total colsum -> vtxb [1, D+1] bf16; element D = S exactly.
                # fp32 matmuls on the otherwise-idle PE during the load window.
                vt_ps = ps_s.tile([1, 4, D + 1], f32, tag="ps")
                for m in range(8):
                    nc.tensor.matmul(
                        vt_ps,
                        ones_col,
                        vh[:, 4 * m : 4 * (m + 1), :],
                        start=(m == 0),
                        stop=(m == 7),
                    )
                vt4 = vtp.tile([1, 4 * (D + 1)], f32, tag="vt4")
                nc.scalar.copy(out=vt4, in_=vt_ps.rearrange("p a b -> p (a b)"))
                vt2 = vtp.tile([1, 2 * (D + 1)], f32, tag="vt2")
                nc.vector.tensor_add(
                    vt2,
                    vt4[:, 0 : 2 * (D + 1)],
                    vt4[:, 2 * (D + 1) : 4 * (D + 1)],
                )
                vtxb = vtp.tile([1, D + 1], bf16, tag="vtxb")
                nc.vector.tensor_add(
                    vtxb, vt2[:, 0 : D + 1], vt2[:, D + 1 : 2 * (D + 1)]
                )

                for j in range(NPAIR):
                    c0 = 2 * j
                    # -- 4 per-chunk transposes into one PSUM bank --
                    pt = ps_t.tile([64, 512], bf16, tag="pt")
                    nc.tensor.transpose(pt[:, 0:128], qhb[:, c0, :], identb)
                    nc.tensor.transpose(pt[:, 128:256], qhb[:, c0 + 1, :], identb)
                    nc.tensor.transpose(pt[:, 256:384], khb[:, c0, :], identb)
                    nc.tensor.transpose(pt[:, 384:512], khb[:, c0 + 1, :], identb)
                    tsb = tsbs[j % NT]
                    nc.vector.tensor_copy(out=tsb[0:64, :], in_=pt)

                    for cc in range(2):
                        c = c0 + cc
                        # -- scores S^T[k, q] with mask rows: cross -> -M^2 --
                        pss = ps_s.tile(
                            [128, 128], f32, tag="ps", name=f"ps_{h}_{j}_{cc}"
                        )
                        nc.tensor.matmul(
                            pss,
                            tsb[:, 256 + 128 * cc : 256 + 128 * (cc + 1)],
                            tsb[:, 128 * cc : 128 * (cc + 1)],
                            start=True,
                            stop=True,
                        )
                        # -- E^T = exp(S^T/8): exact 0 on cross quadrants --
                        et = work.tile([128, 128], bf16, tag="et")
                        nc.scalar.activation(out=et, in_=pss, func=EXP, scale=SCALE)

                        # -- num|denom --
                        po = ps_o.tile(
                            [128, D + 1], f32, tag="po", name=f"po_{h}_{j}_{cc}"
                        )
                        nc.tensor.matmul(
                            po, et, vhb[:, c, :], start=True, stop=False
                        )
                        nc.tensor.matmul(
                            po, negblk, vhb[:, c, :], start=False, stop=False
                        )
                        nc.tensor.matmul(
                            po, ones_row, vtxb, start=False, stop=True
                        )

                        # -- out = num * (1/denom) --
                        rcp = work.tile([128, 1], f32, tag="rcp")
                        nc.vector.reciprocal(out=rcp, in_=po[:, D : D + 1])
                        if cc == 0:
                            nc.vector.tensor_scalar_mul(
                                oh[:, c, :], po[:, 0:D], rcp
                            )
                        else:
                            nc.scalar.activation(
                                out=oh[:, c, :], in_=po[:, 0:D], func=COPY,
                                scale=rcp,
                            )

                # stores go out on the Scalar engine's HWDGE queue so the
                # next head's loads are not stuck behind them on SyncE
                for quarter in range(4):
                    hs = slice(quarter * (NCHUNK // 4), (quarter + 1) * (NCHUNK // 4))
                    nc.scalar.dma_start(
                        out=o_d[h].rearrange("(c p) d -> p c d", p=128)[:, hs, :],
                        in_=oh[:, hs, :],
                    )

    nc.compile()
    return nc


def _get_compiled():
    if "nc" not in _CACHE:
        _CACHE["nc"] = _build_bass()
    return _CACHE["nc"]


def make_in_maps(query, key, value):
    q = np.ascontiguousarray(np.asarray(query).reshape(H, S, D), dtype=np.float32)
    k = np.ascontiguousarray(np.asarray(key).reshape(H, S, D), dtype=np.float32)
    v = np.ascontiguousarray(np.asarray(value).reshape(H, S, D), dtype=np.float32)
    in_maps = []
    for i in range(NCORES):
        sl = slice(i * HPC, (i + 1) * HPC)
        in_maps.append(
            {
                "query": np.ascontiguousarray(q[sl]),
                "key": np.ascontiguousarray(k[sl]),
                "value": np.ascontiguousarray(v[sl]),
            }
        )
    return in_maps


def run_spmd(in_maps, **kwargs):
    from concourse.bass_utils import run_bass_kernel_spmd

    nc = _get_compiled()
    return run_bass_kernel_spmd(nc, in_maps, core_ids=list(range(NCORES)), **kwargs)


def assemble(res):
    outs = [res.results[i]["out"] for i in range(NCORES)]
    return np.concatenate(outs, axis=0).reshape(1, H, S, D).astype(np.float32)


def kernel(query: np.ndarray, key: np.ndarray, value: np.ndarray) -> np.ndarray:
    return assemble(run_spmd(make_in_maps(query, key, value)))

